# revision 16
# baseline (speedup 1.0000x reference)
"""3x3 median filter (zero-padded) on TRN2, 8 NeuronCores, bf16 datapath.

Input  x: (32, 3, 512, 512) float32
Output  : (32, 3, 512, 512) float32.

Accuracy: the median network only ever SELECTS one of its 9 inputs (min/max
ops create no new values), so the device-side bf16 result equals the bf16
rounding of the element that is the median of the rounded window. Order
statistics are 1-Lipschitz under sup-norm perturbation, so end-to-end error
is <= 2^-8 relative -- far inside the 2e-2 gate. Measured 3.4e-3.

Strategy
--------
Pure data parallel: batch dim sharded 4-per-core across 8 cores; per core
12 images (4 batch x 3 chan) in 2 groups of 6 images x 2 vertical halves.

bf16 doubles DVE tensor_tensor throughput (2x_1P perf mode) but ONLY for
unit-stride 4-byte-aligned access patterns, so the horizontal stage is
restructured from the fp32 baseline's stride-2 parity tricks into dense
shifted-field form:

  stage 1 (vertical, 5 TT/elem, all dense): row-pair (qmn,qmx) shared by
  both output-row parities, flat over the whole grid.

  stage 2 (horizontal, 12 TT/elem, all dense + aligned): per field
  F in {MN,MD,MX} build s1F[j]=F[j+1] (the ONLY odd-element shift, done
  as a ScalarE copy -- ACT is otherwise idle), then
     P[j]  = op(F[j], s1F[j])          # aligned TT, 2x
     R[j]  = op(P[j], F[j+2])          # +2 elems = 4B-aligned shift, 2x
  R[j] = sliding-3 result centered at col j+1; the final med3(Rmn,Rmd,Rmx)
  writes into an OUT grid whose per-image segment holds col c at position
  c+1, so the wide write starts at even offset 2 and the DMA store (which
  doesn't care about alignment) un-shifts.

  Output cols 0 and 511 (windows containing the zero pad column): ScalarE
  gathers P/Q values at grid positions {0,510} of both parities into one
  dense tile; 6 DVE ops of width 4*nimg + 2 per-parity writes into the
  OUT grids finish them (instead of 14 strided tiny ops).

Grid: per-image segment width 514 (even -> every segment start keeps 4B
parity). Segment positions 512..513 are scratch: stage-1 ops run flat over
the whole grid and compute garbage there; no stored output reads a garbage
lane (out cols 0/511 come from the boundary path).

Image rows 0 and 511 (windows contain the zero pad row): one small
24-partition pass. Its loads are issued up front (tiny); its compute is
issued LAST so it fills the DVE-idle tail while the final block's output
stores drain.

Stage-2 temp tiles alias aggressively (MN1<-Pmn, MX1<-Qmn, TF<-Qmx,
tmd<-Pmx, stage-1 t_o/t_e <- Rmn/Rmd buffers): DVE issue order makes every
WAR safe, and it buys the SBUF headroom for 6-image groups.

Engine budget per core (theory): DVE 17 TT/elem at 2x ~= 220us busy, ACT
~80us, DMA ~19MB. Loads on the SP+ACT HWDGE queues, stores on the GpSimd
SWDGE queue.
"""
import sys

if "/opt/trn_rl_repo" not in sys.path:
    sys.path.insert(0, "/opt/trn_rl_repo")

import numpy as np
import ml_dtypes
import concourse.bacc as bacc
import concourse.mybir as mybir
import concourse.tile as tile
from concourse import bass_utils

B, C, H, W = 32, 3, 512, 512
N_CORES = 8
B_PER = B // N_CORES          # 4 batches per core
NIMG = B_PER * C              # 12 images per core
GIMG = 6                      # images per tile group
PW = W + 2                    # per-image grid width (514, even)
FP = GIMG * PW                # flat grid width of row tiles (3084)
HH = H // 2                   # 256 rows per vertical half
P = 128                       # partitions = row pairs per half
NE = 2 * NIMG                 # partitions in the edge-rows pass (24)

BF16 = mybir.dt.bfloat16
MIN = mybir.AluOpType.min
MAX = mybir.AluOpType.max

_PROGRAM = None


def _seg(T, npart, nimg):
    """[npart, nimg, 514] per-image-segment view."""
    return T[:].rearrange("p (i w) -> p i w", w=PW)[0:npart, 0:nimg]


def _stage2_copies(nc, pm, MN, MD, MX, npart, nimg, pref):
    """ScalarE: the one odd-element shift per field, s1F[j] = F[j+1]."""
    NF = nimg * PW
    s1 = {}
    for name, F in (("MN", MN), ("MD", MD), ("MX", MX)):
        T = pm.tile([P, FP], BF16, tag=f"s1{name}", name=f"{pref}s1{name}")
        nc.scalar.copy(T[:][0:npart, 0 : NF - 1], F[:][0:npart, 1:NF])
        s1[name] = T
    return s1


def _stage2_compute(nc, pm, MN, MD, MX, s1, OUT, npart, nimg, pref, bnd=None):
    """DVE: dense aligned sliding-3 per field + final med3 -> OUT grid.
    OUT per-image position c+1 holds output col c (cols 1..510 here).
    If bnd is given (main blocks), ScalarE gathers the P/Q boundary
    columns into bnd[parity] for the deferred batched boundary pass;
    otherwise (edge pass) boundary cols are done inline."""
    NF = nimg * PW
    NI = NF - 2
    fl = lambda T, a, b: T[:][0:npart, a:b]

    def t2(tag):
        return pm.tile([P, FP], BF16, tag=tag, name=f"{pref}{tag}")

    Pmn, Pmx, Qmn, Qmx = t2("Pmn"), t2("Pmx"), t2("Qmn"), t2("Qmx")
    Rmn, Rmd, Rmx = t2("Rmn"), t2("Rmd"), t2("Rmx")
    # aliases -- disjoint lifetimes given the op order below
    tmd = pm.tile([P, FP], BF16, tag="Pmx", name=f"{pref}tmd")
    MN1 = pm.tile([P, FP], BF16, tag="Pmn", name=f"{pref}MN1")
    MX1 = pm.tile([P, FP], BF16, tag="Qmn", name=f"{pref}MX1")
    TF = pm.tile([P, FP], BF16, tag="Qmx", name=f"{pref}TF")

    tt = nc.vector.tensor_tensor
    # adjacent-column pairs (j, j+1)
    tt(fl(Pmn, 0, NI), fl(MN, 0, NI), fl(s1["MN"], 0, NI), op=MAX)
    tt(fl(Qmn, 0, NI), fl(MD, 0, NI), fl(s1["MD"], 0, NI), op=MIN)
    tt(fl(Qmx, 0, NI), fl(MD, 0, NI), fl(s1["MD"], 0, NI), op=MAX)
    tt(fl(Pmx, 0, NI), fl(MX, 0, NI), fl(s1["MX"], 0, NI), op=MIN)

    gv = lambda T: _seg(T, npart, nimg)[:, :, 0:511:510]
    if bnd is not None:
        # ScalarE pulls the {0,510} boundary columns out now so the P/Q
        # buffers can be reused (aliases above) and the boundary math can
        # run batched across both parities later
        for nm, T in (("Pmn", Pmn), ("Pmx", Pmx), ("Qmn", Qmn), ("Qmx", Qmx)):
            nc.scalar.copy(bnd[nm], gv(T))

    # close the window with the +2 (aligned) shift: R[j] ~ output col j+1
    tt(fl(Rmn, 0, NI), fl(Pmn, 0, NI), fl(MN, 2, NF), op=MAX)
    tt(fl(Rmx, 0, NI), fl(Pmx, 0, NI), fl(MX, 2, NF), op=MIN)
    tt(fl(tmd, 0, NI), fl(Qmx, 0, NI), fl(MD, 2, NF), op=MIN)
    tt(fl(Rmd, 0, NI), fl(Qmn, 0, NI), fl(tmd, 0, NI), op=MAX)
    # med3(Rmn, Rmd, Rmx); last op writes output cols 1..510 at grid
    # positions 2..511 (even start -> stays in 2x mode)
    tt(fl(MN1, 0, NI), fl(Rmn, 0, NI), fl(Rmd, 0, NI), op=MIN)
    tt(fl(MX1, 0, NI), fl(Rmn, 0, NI), fl(Rmd, 0, NI), op=MAX)
    tt(fl(TF, 0, NI), fl(MX1, 0, NI), fl(Rmx, 0, NI), op=MIN)
    ov = _seg(OUT, npart, nimg)[:, :, 2:512]
    tt(ov, _seg(MN1, npart, nimg)[:, :, 0:510],
       _seg(TF, npart, nimg)[:, :, 0:510], op=MAX)

    if bnd is None:
        # inline boundary (last block): A=max(P,0), C=min(Pmx,0),
        # B=max(Qmn,min(Qmx,0)), med3 -> OUT positions {1, 512}
        bt = lambda tag: pm.tile([P, 2 * GIMG], BF16, tag=f"i{tag}b",
                                 name=f"{pref}i{tag}b")
        bv = lambda T: T[:].rearrange("p (i c) -> p i c", c=2)[0:npart, 0:nimg]
        BA, BB, BC = bt("BA"), bt("BB"), bt("BC")
        B1, B2, B3 = bt("B1"), bt("B2"), bt("B3")
        nc.vector.tensor_scalar_max(bv(BA), gv(Pmn), 0.0)
        nc.vector.tensor_scalar_min(bv(BC), gv(Pmx), 0.0)
        nc.vector.scalar_tensor_tensor(bv(BB), gv(Qmx), 0.0, gv(Qmn),
                                       op0=MIN, op1=MAX)
        tt(bv(B1), bv(BA), bv(BB), op=MIN)
        tt(bv(B2), bv(BA), bv(BB), op=MAX)
        tt(bv(B3), bv(B2), bv(BC), op=MIN)
        obv = _seg(OUT, npart, nimg)[:, :, 1:513:511]
        tt(obv, bv(B1), bv(B3), op=MAX)


def _boundary_batch(nc, pm, BP, OUT_o, OUT_e):
    """Output cols 0 and 511 for both parities in one dense pass.
    BP[nm] tiles are [P, 2*GIMG*2] with layout (parity, img, col)."""
    tt = nc.vector.tensor_tensor
    bt = lambda tag: pm.tile([P, 4 * GIMG], BF16, tag=f"{tag}b", name=f"{tag}b")
    BA, BB, BC = bt("BA"), bt("BB"), bt("BC")
    B1, B2, B3 = bt("B1"), bt("B2"), bt("B3")
    nc.vector.tensor_scalar_max(BA[:], BP["Pmn"][:], 0.0)
    nc.vector.tensor_scalar_min(BC[:], BP["Pmx"][:], 0.0)
    nc.vector.scalar_tensor_tensor(BB[:], BP["Qmx"][:], 0.0, BP["Qmn"][:],
                                   op0=MIN, op1=MAX)
    tt(B1[:], BA[:], BB[:], op=MIN)
    tt(B2[:], BA[:], BB[:], op=MAX)
    tt(B3[:], B2[:], BC[:], op=MIN)
    pv = lambda T, h: T[:].rearrange("p (h i c) -> p h i c", h=2, c=2)[:, h]
    for h, OUT in ((0, OUT_o), (1, OUT_e)):
        obv = _seg(OUT, P, GIMG)[:, :, 1:513:511]
        tt(obv, pv(B1, h), pv(B3, h), op=MAX)


def _block(nc, pio, pm, xh, oh, g, half, first=False, last=False):
    """One vertical half of one image group: odd output rows r0+1..r0+255,
    even rows r0+2..r0+256 (halves overlap by two rows so every DMA is a
    full 128-partition transfer). Rows 0 and 511 via the edge pass.
    first=True: loads and stage 1 split into two 3-image chunks so the DVE
    starts after ~1.5MB has landed instead of ~3MB (cold-start only).
    last=True: the two output stores go to the by-then-idle HWDGE queues
    so the final drain overlaps the edge pass."""
    r0 = 0 if half == 0 else H - HH - 2
    i0 = GIMG * g

    E = pio.tile([P, FP], BF16, tag="E", name="E", bufs=2)
    O = pio.tile([P, FP], BF16, tag="O", name="O", bufs=2)
    E_sh = pio.tile([P, FP], BF16, tag="E_sh", name="E_sh", bufs=2)
    O_sh2 = pio.tile([P, FP], BF16, tag="O_sh2", name="O_sh2", bufs=2)

    # scratch cols 512..513 of each segment are read by the flat stage-1
    # ops but never loaded: define them so no lane is uninitialized
    for T in (E, O, E_sh, O_sh2):
        nc.gpsimd.memset(_seg(T, P, GIMG)[:, :, W:PW], 0.0)

    def loads(ia, ib):
        lv = lambda T: _seg(T, P, GIMG)[:, ia:ib, 0:W]
        im = lambda r_lo: xh[
            r_lo : min(r_lo + 2 * P, H) : 2, i0 + ia : i0 + ib, :
        ]
        # queue order matters (HWDGE queues are FIFOs): the (O, E_sh) pair
        # feeds the first op of the block, so those loads go first
        nc.sync.dma_start(lv(E_sh), im(r0 + 2))     # rows r0+2p+2
        nc.scalar.dma_start(lv(O), im(r0 + 1))      # rows r0+2p+1
        nc.sync.dma_start(lv(E), im(r0))            # rows r0+2p
        nc.scalar.dma_start(lv(O_sh2), im(r0 + 3))  # rows r0+2p+3

    # stage 1: shared pair = (O, E_sh) = rows (2p+1, 2p+2); flat ops
    qmn = pm.tile([P, FP], BF16, tag="qmn", name="qmn")
    qmx = pm.tile([P, FP], BF16, tag="qmx", name="qmx")
    tt = nc.vector.tensor_tensor

    def fld(tag):
        return pm.tile([P, FP], BF16, tag=tag, name=tag)

    MN_o, MD_o, MX_o = fld("MN_o"), fld("MD_o"), fld("MX_o")
    MN_e, MD_e, MX_e = fld("MN_e"), fld("MD_e"), fld("MX_e")
    # stage-1 temps alias stage-2 R slots (dead before those are written)
    t_o = pm.tile([P, FP], BF16, tag="Rmn", name="t_o")
    t_e = pm.tile([P, FP], BF16, tag="Rmd", name="t_e")

    def stage1(ia, ib):
        fv = lambda T: T[:][:, ia * PW : ib * PW]
        tt(fv(qmn), fv(O), fv(E_sh), op=MIN)
        tt(fv(qmx), fv(O), fv(E_sh), op=MAX)
        # odd output rows r0+2p+1: pair + E (row r0+2p)
        tt(fv(MN_o), fv(qmn), fv(E), op=MIN)
        tt(fv(MX_o), fv(qmx), fv(E), op=MAX)
        tt(fv(t_o), fv(qmx), fv(E), op=MIN)
        tt(fv(MD_o), fv(qmn), fv(t_o), op=MAX)
        # even output rows r0+2p+2: pair + O_sh2 (row r0+2p+3)
        tt(fv(MN_e), fv(qmn), fv(O_sh2), op=MIN)
        tt(fv(MX_e), fv(qmx), fv(O_sh2), op=MAX)
        tt(fv(t_e), fv(qmx), fv(O_sh2), op=MIN)
        tt(fv(MD_e), fv(qmn), fv(t_e), op=MAX)

    if first:
        # tiny first chunk: the DVE starts after ~0.25MB instead of ~3MB
        loads(0, 1)
        loads(1, GIMG)
        stage1(0, 1)
        stage1(1, GIMG)
    else:
        loads(0, GIMG)
        stage1(0, GIMG)

    # boundary gather tiles: layout (parity, img, col{0,510})
    BP = {
        nm: pm.tile([P, 4 * GIMG], BF16, tag=f"BP{nm}", name=f"BP{nm}")
        for nm in ("Pmn", "Pmx", "Qmn", "Qmx")
    }
    hv = lambda nm, h: BP[nm][:].rearrange("p (h i c) -> p h i c", h=2, c=2)[:, h]

    OUT_o = pio.tile([P, FP], BF16, tag="OUT_o", name="OUT_o")
    OUT_e = pio.tile([P, FP], BF16, tag="OUT_e", name="OUT_e")
    out_img = lambda r_lo: oh[r_lo : min(r_lo + 2 * P, H) : 2, i0 : i0 + GIMG, :]
    ostore = lambda T: _seg(T, P, GIMG)[:, :, 1:513]

    s1_o = _stage2_copies(nc, pm, MN_o, MD_o, MX_o, P, GIMG, "o_")
    _stage2_compute(nc, pm, MN_o, MD_o, MX_o, s1_o, OUT_o, P, GIMG, "o_",
                    bnd={nm: hv(nm, 0) for nm in BP})
    s1_e = _stage2_copies(nc, pm, MN_e, MD_e, MX_e, P, GIMG, "e_")
    _stage2_compute(nc, pm, MN_e, MD_e, MX_e, s1_e, OUT_e, P, GIMG, "e_",
                    bnd={nm: hv(nm, 1) for nm in BP})
    _boundary_batch(nc, pm, BP, OUT_o, OUT_e)

    if last:
        # HWDGE queues are idle by now (all loads issued); their stores
        # drain while the edge pass runs, shrinking the end-of-kernel tail
        nc.sync.dma_start(out_img(r0 + 1), ostore(OUT_o))
        nc.scalar.dma_start(out_img(r0 + 2), ostore(OUT_e))
    else:
        # stores on the SWDGE queue so they never block later blocks' loads
        nc.gpsimd.dma_start(out_img(r0 + 1), ostore(OUT_o))
        nc.gpsimd.dma_start(out_img(r0 + 2), ostore(OUT_e))


def _edge_loads(nc, pio, xi):
    """Loads for image rows 0 and 511 (tiny, partial-partition): issued up
    front so the end-of-kernel edge compute never waits on DMA."""
    R0 = pio.tile([NE, PW], BF16, tag="R0", name="R0")
    R1 = pio.tile([NE, PW], BF16, tag="R1", name="R1")
    for T in (R0, R1):
        nc.gpsimd.memset(T[:][0:NE, W:PW], 0.0)
    nc.sync.dma_start(R0[:][0:NIMG, 0:W], xi[:, 0, :])
    nc.scalar.dma_start(R1[:][0:NIMG, 0:W], xi[:, 1, :])
    nc.sync.dma_start(R0[:][NIMG:NE, 0:W], xi[:, H - 1, :])
    nc.scalar.dma_start(R1[:][NIMG:NE, 0:W], xi[:, H - 2, :])
    return R0, R1


def _edge_compute(nc, pio, pm, oi, R0, R1):
    """Rows 0 and 511 (windows contain the zero pad row), 24 partitions:
    p 0..11 = row 0 of image p; p 12..23 = row 511 of image p-12. Runs
    last, in the shadow of the final block's output stores."""
    rmn = pm.tile([NE, PW], BF16, tag="rmn", name="rmn")
    rmx = pm.tile([NE, PW], BF16, tag="rmx", name="rmx")
    nc.vector.tensor_tensor(rmn[:], R0[:], R1[:], op=MIN)
    nc.vector.tensor_tensor(rmx[:], R0[:], R1[:], op=MAX)

    # vertical sort3 with the zero pad row: min/max vs 0, med=max(mn,min(mx,0))
    MN0 = pm.tile([NE, PW], BF16, tag="eMN", name="eMN")
    MD0 = pm.tile([NE, PW], BF16, tag="eMD", name="eMD")
    MX0 = pm.tile([NE, PW], BF16, tag="eMX", name="eMX")
    nc.vector.tensor_scalar_min(MN0[:], rmn[:], 0.0)
    nc.vector.tensor_scalar_max(MX0[:], rmx[:], 0.0)
    nc.vector.scalar_tensor_tensor(MD0[:], rmx[:], 0.0, rmn[:], op0=MIN, op1=MAX)

    s1 = {}
    for name, F in (("MN", MN0), ("MD", MD0), ("MX", MX0)):
        T = pm.tile([NE, PW], BF16, tag=f"es1{name}", name=f"es1{name}")
        nc.scalar.copy(T[:][0:NE, 0 : PW - 1], F[:][0:NE, 1:PW])
        s1[name] = T

    OUT0 = pio.tile([NE, PW], BF16, tag="OUT0", name="OUT0")
    _stage2_compute_small(nc, pm, MN0, MD0, MX0, s1, OUT0)
    # the kernel's very last stores: HWDGE (faster first-byte than SWDGE)
    nc.sync.dma_start(oi[:, 0, :], OUT0[:][0:NIMG, 1:513])
    nc.scalar.dma_start(oi[:, H - 1, :], OUT0[:][NIMG:NE, 1:513])


def _stage2_compute_small(nc, pm, MN, MD, MX, s1, OUT):
    """Same dataflow as _stage2_compute on [NE, PW] tiles (nimg=1)."""
    NF = PW
    NI = NF - 2
    fl = lambda T, a, b: T[:][0:NE, a:b]

    def t2(tag):
        return pm.tile([NE, PW], BF16, tag=f"e{tag}", name=f"e{tag}")

    Pmn, Pmx, Qmn, Qmx = t2("Pmn"), t2("Pmx"), t2("Qmn"), t2("Qmx")
    tmd, Rmn, Rmd, Rmx = t2("tmd"), t2("Rmn"), t2("Rmd"), t2("Rmx")
    MN1, MX1, TF = t2("MN1"), t2("MX1"), t2("TF")

    tt = nc.vector.tensor_tensor
    tt(fl(Pmn, 0, NI), fl(MN, 0, NI), fl(s1["MN"], 0, NI), op=MAX)
    tt(fl(Qmn, 0, NI), fl(MD, 0, NI), fl(s1["MD"], 0, NI), op=MIN)
    tt(fl(Qmx, 0, NI), fl(MD, 0, NI), fl(s1["MD"], 0, NI), op=MAX)
    tt(fl(Pmx, 0, NI), fl(MX, 0, NI), fl(s1["MX"], 0, NI), op=MIN)
    tt(fl(Rmn, 0, NI), fl(Pmn, 0, NI), fl(MN, 2, NF), op=MAX)
    tt(fl(tmd, 0, NI), fl(Qmx, 0, NI), fl(MD, 2, NF), op=MIN)
    tt(fl(Rmd, 0, NI), fl(Qmn, 0, NI), fl(tmd, 0, NI), op=MAX)
    tt(fl(Rmx, 0, NI), fl(Pmx, 0, NI), fl(MX, 2, NF), op=MIN)
    tt(fl(MN1, 0, NI), fl(Rmn, 0, NI), fl(Rmd, 0, NI), op=MIN)
    tt(fl(MX1, 0, NI), fl(Rmn, 0, NI), fl(Rmd, 0, NI), op=MAX)
    tt(fl(TF, 0, NI), fl(MX1, 0, NI), fl(Rmx, 0, NI), op=MIN)
    tt(fl(OUT, 2, 512), fl(MN1, 0, 510), fl(TF, 0, 510), op=MAX)

    gv = lambda T: T[:][0:NE, 0:511:510]
    bt = lambda tag: pm.tile([NE, 2], BF16, tag=f"e{tag}b", name=f"e{tag}b")
    BA, BB, BC = bt("BA"), bt("BB"), bt("BC")
    B1, B2, B3 = bt("B1"), bt("B2"), bt("B3")
    nc.vector.tensor_scalar_max(BA[:], gv(Pmn), 0.0)
    nc.vector.tensor_scalar_min(BC[:], gv(Pmx), 0.0)
    nc.vector.scalar_tensor_tensor(BB[:], gv(Qmx), 0.0, gv(Qmn), op0=MIN, op1=MAX)
    tt(B1[:], BA[:], BB[:], op=MIN)
    tt(B2[:], BA[:], BB[:], op=MAX)
    tt(B3[:], B2[:], BC[:], op=MIN)
    tt(OUT[:][0:NE, 1:513:511], B1[:], B3[:], op=MAX)


def build_program():
    nc = bacc.Bacc(
        "TRN2", target_bir_lowering=False, debug=False, num_devices=N_CORES
    )
    x_d = nc.dram_tensor("x", [B_PER, C, H, W], BF16, kind="ExternalInput").ap()
    o_d = nc.dram_tensor("out", [B_PER, C, H, W], BF16, kind="ExternalOutput").ap()
    xh = x_d.rearrange("b c h w -> h (b c) w")  # [512, 12, 512]
    oh = o_d.rearrange("b c h w -> h (b c) w")
    xi = x_d.rearrange("b c h w -> (b c) h w")  # [12, 512, 512]
    oi = o_d.rearrange("b c h w -> (b c) h w")

    with tile.TileContext(nc) as tc:
        with (
            tc.tile_pool(name="io", bufs=1) as pio,
            tc.tile_pool(name="mid", bufs=1) as pm,
        ):
            nb = 2 * (NIMG // GIMG)
            _block(nc, pio, pm, xh, oh, 0, 0, first=True)
            # edge loads are tiny; edge COMPUTE runs last, in the shadow of
            # the final block's output stores
            R0, R1 = _edge_loads(nc, pio, xi)
            for i in range(1, nb):
                _block(nc, pio, pm, xh, oh, i // 2, i % 2, last=(i == nb - 1))
            _edge_compute(nc, pio, pm, oi, R0, R1)
    nc.compile()
    return nc


def _get_program():
    global _PROGRAM
    if _PROGRAM is None:
        _PROGRAM = build_program()
    return _PROGRAM


def make_in_maps(x: np.ndarray):
    xb = np.ascontiguousarray(x).astype(ml_dtypes.bfloat16)
    return [{"x": xb[k * B_PER : (k + 1) * B_PER]} for k in range(N_CORES)]


def kernel(**inputs) -> np.ndarray:
    x = np.asarray(inputs["x"], dtype=np.float32)
    assert x.shape == (B, C, H, W), x.shape
    nc = _get_program()
    res = bass_utils.run_bass_kernel_spmd(
        nc, make_in_maps(x), core_ids=list(range(N_CORES))
    )
    out = np.concatenate(
        [np.asarray(res.results[k]["out"]) for k in range(N_CORES)], axis=0
    )
    return out.astype(np.float32)


# revision 17
# speedup vs baseline: 1.0063x; 1.0063x over previous
"""3x3 median filter (zero-padded) on TRN2, 8 NeuronCores, bf16 datapath.

Input  x: (32, 3, 512, 512) float32
Output  : (32, 3, 512, 512) float32.

Accuracy: the median network only ever SELECTS one of its 9 inputs (min/max
ops create no new values), so the device-side bf16 result equals the bf16
rounding of the element that is the median of the rounded window. Order
statistics are 1-Lipschitz under sup-norm perturbation, so end-to-end error
is <= 2^-8 relative -- far inside the 2e-2 gate. Measured 3.4e-3.

Strategy
--------
Pure data parallel: batch dim sharded 4-per-core across 8 cores; per core
12 images (4 batch x 3 chan) in 2 groups of 6 images x 2 vertical halves.

bf16 doubles DVE tensor_tensor throughput (2x_1P perf mode) but ONLY for
unit-stride 4-byte-aligned access patterns, so the horizontal stage is
restructured from the fp32 baseline's stride-2 parity tricks into dense
shifted-field form. Per field F in {MN,MD,MX}: s1F[j]=F[j+1] is the ONLY
odd-element shift (a ScalarE copy -- ACT is otherwise idle), then
   P[j] = op(F[j], s1F[j])          # aligned TT, 2x
   R[j] = op(P[j], F[j+2])          # +2 elems = 4B-aligned shift, 2x
R[j] = sliding-3 result centered at col j+1; the final med3(Rmn,Rmd,Rmx)
writes into an OUT grid whose per-image segment holds col c at position
c+1, so the wide write starts at even offset 2 and the DMA store (which
doesn't care about alignment) un-shifts.

Both row parities' fields live in ONE fused [128, 12*514] tile (odd-parity
images = segments 0..5, even = 6..11), so stage 2 is 12 double-width ops
per block instead of 24 -- per-op overhead (58-cycle issue + ~90ns DRAIN)
is the only thing that changes, the streamed cycles are identical. Output
cols 0 and 511 (windows containing the zero pad column) are 7 tiny ops on
gathered grid positions {0,510} across all 12 segments.

Grid: per-image segment width 514 (even -> every segment start keeps 4B
parity). Segment positions 512..513 are scratch: stage-1 ops run flat over
the whole grid and compute garbage there; no stored output reads a garbage
lane (out cols 0/511 come from the boundary path).

SBUF fits via aggressive aliasing with DVE-program-order-safe lifetimes:
stage-1's qmn/qmx/t_o/t_e live in halves of stage-2's Pmx/Qmx slots;
stage-2's Rmx/tmd/Rmd overwrite the dead MN/MX/MD field buffers; MN1/MX1/TF
overwrite Pmn/Qmn/Qmx. Stage-1 emits fields in MN,MX,MD order and ACT
copies s1MN,s1MX,s1MD in that order so every copy lands before the DVE
needs it, with no stall.

Image rows 0 and 511 (windows contain the zero pad row): one small
24-partition pass issued LAST so it fills the DVE-idle tail while the
final block's output stores (sent to the idle HWDGE queues) drain. Its
tiny loads are issued up front. Block 0's loads+stage-1 ramp up in 1+2+3
image chunks sized to the ~250GB/s strided-row-gather DMA rate, so the
DVE starts ~2us after the first 0.25MB lands instead of waiting for 3MB.

Engine budget per core: DVE 17 TT/elem at 2x ~= 236us busy (the floor for
this decomposition), ACT ~75us, DMA ~19MB. Loads on the SP+ACT HWDGE
queues, mid-kernel stores on the GpSimd SWDGE queue.
"""
import sys

if "/opt/trn_rl_repo" not in sys.path:
    sys.path.insert(0, "/opt/trn_rl_repo")

import numpy as np
import ml_dtypes
import concourse.bacc as bacc
import concourse.mybir as mybir
import concourse.tile as tile
from concourse import bass_utils

B, C, H, W = 32, 3, 512, 512
N_CORES = 8
B_PER = B // N_CORES          # 4 batches per core
NIMG = B_PER * C              # 12 images per core
GIMG = 6                      # images per tile group
PW = W + 2                    # per-image grid width (514, even)
FP = GIMG * PW                # half (one parity) grid width (3084)
FP2 = 2 * FP                  # fused two-parity grid width (6168)
NSEG = 2 * GIMG               # segments in a fused tile (12)
HH = H // 2                   # 256 rows per vertical half
P = 128                       # partitions = row pairs per half
NE = 2 * NIMG                 # partitions in the edge-rows pass (24)

BF16 = mybir.dt.bfloat16
MIN = mybir.AluOpType.min
MAX = mybir.AluOpType.max

_PROGRAM = None


def _seg(T, npart, nseg):
    """[npart, nseg, 514] per-image-segment view."""
    return T[:].rearrange("p (i w) -> p i w", w=PW)[0:npart, 0:nseg]


def _stage2_fused(nc, pm, MN, MD, MX, OUT):
    """Both parities of stage 2 in double-width ops on the fused grids.
    ScalarE does the three s1 shifts (issued in MN,MX,MD order to match
    the DVE's consumption order). Aliases overwrite only dead buffers."""
    NI = FP2 - 2
    fl = lambda T, a, b: T[:][:, a:b]

    s1 = {}
    for nm, F in (("MN", MN), ("MX", MX), ("MD", MD)):
        T = pm.tile([P, FP2], BF16, tag=f"s1{nm}", name=f"s1{nm}")
        nc.scalar.copy(T[:][:, 0 : FP2 - 1], F[:][:, 1:FP2])
        s1[nm] = T

    def t2(tag):
        return pm.tile([P, FP2], BF16, tag=tag, name=tag)

    def alias(tag, name):
        return pm.tile([P, FP2], BF16, tag=tag, name=name)

    Pmn, Pmx, Qmn, Qmx, Rmn = t2("Pmn"), t2("Pmx"), t2("Qmn"), t2("Qmx"), t2("Rmn")
    Rmx = alias("fMN", "Rmx")   # MN field dead after Rmn
    tmd = alias("fMX", "tmd")   # MX field dead after Rmx
    Rmd = alias("fMD", "Rmd")   # MD field dead after tmd
    MN1 = alias("Pmn", "MN1")
    MX1 = alias("Qmn", "MX1")
    TF = alias("Qmx", "TF")

    tt = nc.vector.tensor_tensor
    gv = lambda T: _seg(T, P, NSEG)[:, :, 0:511:510]
    bt = lambda tag: pm.tile([P, 2 * NSEG], BF16, tag=f"{tag}b", name=f"{tag}b")
    bv = lambda T: T[:].rearrange("p (i c) -> p i c", c=2)[0:P, 0:NSEG]
    BA, BB, BC = bt("BA"), bt("BB"), bt("BC")
    B1, B2, B3 = bt("B1"), bt("B2"), bt("B3")

    tt(fl(Pmn, 0, NI), fl(MN, 0, NI), fl(s1["MN"], 0, NI), op=MAX)
    tt(fl(Pmx, 0, NI), fl(MX, 0, NI), fl(s1["MX"], 0, NI), op=MIN)
    nc.vector.tensor_scalar_max(bv(BA), gv(Pmn), 0.0)
    nc.vector.tensor_scalar_min(bv(BC), gv(Pmx), 0.0)
    tt(fl(Rmn, 0, NI), fl(Pmn, 0, NI), fl(MN, 2, FP2), op=MAX)
    tt(fl(Rmx, 0, NI), fl(Pmx, 0, NI), fl(MX, 2, FP2), op=MIN)
    tt(fl(Qmn, 0, NI), fl(MD, 0, NI), fl(s1["MD"], 0, NI), op=MIN)
    tt(fl(Qmx, 0, NI), fl(MD, 0, NI), fl(s1["MD"], 0, NI), op=MAX)
    nc.vector.scalar_tensor_tensor(bv(BB), gv(Qmx), 0.0, gv(Qmn), op0=MIN, op1=MAX)
    tt(fl(tmd, 0, NI), fl(Qmx, 0, NI), fl(MD, 2, FP2), op=MIN)
    tt(fl(Rmd, 0, NI), fl(Qmn, 0, NI), fl(tmd, 0, NI), op=MAX)
    # med3(Rmn, Rmd, Rmx); last wide op writes output cols 1..510 at grid
    # positions 2..511 (even start -> stays in 2x mode)
    tt(fl(MN1, 0, NI), fl(Rmn, 0, NI), fl(Rmd, 0, NI), op=MIN)
    tt(fl(MX1, 0, NI), fl(Rmn, 0, NI), fl(Rmd, 0, NI), op=MAX)
    tt(fl(TF, 0, NI), fl(MX1, 0, NI), fl(Rmx, 0, NI), op=MIN)
    ov = _seg(OUT, P, NSEG)[:, :, 2:512]
    tt(ov, _seg(MN1, P, NSEG)[:, :, 0:510], _seg(TF, P, NSEG)[:, :, 0:510],
       op=MAX)
    # boundary med3 -> OUT positions {1, 512}
    tt(bv(B1), bv(BA), bv(BB), op=MIN)
    tt(bv(B2), bv(BA), bv(BB), op=MAX)
    tt(bv(B3), bv(B2), bv(BC), op=MIN)
    tt(_seg(OUT, P, NSEG)[:, :, 1:513:511], bv(B1), bv(B3), op=MAX)


def _block(nc, pio, pm, xh, oh, g, half, first=False, last=False):
    """One vertical half of one image group: odd output rows r0+1..r0+255,
    even rows r0+2..r0+256 (halves overlap by two rows so every DMA is a
    full 128-partition transfer). Rows 0 and 511 via the edge pass.
    first=True: loads and stage 1 ramp in 1+2+3 image chunks (cold start).
    last=True: stores go to the by-then-idle HWDGE queues so the final
    drain overlaps the edge pass."""
    r0 = 0 if half == 0 else H - HH - 2
    i0 = GIMG * g

    E = pio.tile([P, FP], BF16, tag="E", name="E")
    O = pio.tile([P, FP], BF16, tag="O", name="O")
    E_sh = pio.tile([P, FP], BF16, tag="E_sh", name="E_sh")
    O_sh2 = pio.tile([P, FP], BF16, tag="O_sh2", name="O_sh2")

    # scratch cols 512..513 of each segment are read by the flat stage-1
    # ops but never loaded: define them so no lane is uninitialized
    for T in (E, O, E_sh, O_sh2):
        nc.gpsimd.memset(_seg(T, P, GIMG)[:, :, W:PW], 0.0)

    def loads(ia, ib):
        lv = lambda T: _seg(T, P, GIMG)[:, ia:ib, 0:W]
        im = lambda r_lo: xh[
            r_lo : min(r_lo + 2 * P, H) : 2, i0 + ia : i0 + ib, :
        ]
        # queue order matters (HWDGE queues are FIFOs): the (O, E_sh) pair
        # feeds the first op of the block, so those loads go first
        nc.sync.dma_start(lv(E_sh), im(r0 + 2))     # rows r0+2p+2
        nc.scalar.dma_start(lv(O), im(r0 + 1))      # rows r0+2p+1
        nc.sync.dma_start(lv(E), im(r0))            # rows r0+2p
        nc.scalar.dma_start(lv(O_sh2), im(r0 + 3))  # rows r0+2p+3

    # stage-1 temps live in halves of stage-2 slots that are written later
    qpair = pm.tile([P, FP2], BF16, tag="Pmx", name="qpair")
    tpair = pm.tile([P, FP2], BF16, tag="Qmx", name="tpair")
    MN2 = pm.tile([P, FP2], BF16, tag="fMN", name="fMN")
    MD2 = pm.tile([P, FP2], BF16, tag="fMD", name="fMD")
    MX2 = pm.tile([P, FP2], BF16, tag="fMX", name="fMX")

    tt = nc.vector.tensor_tensor

    def stage1(ia, ib):
        a, b = ia * PW, ib * PW
        qv = lambda h: qpair[:][:, h * FP + a : h * FP + b]
        tv = lambda h: tpair[:][:, h * FP + a : h * FP + b]
        f = lambda T, h: T[:][:, h * FP + a : h * FP + b]
        sv = lambda T: T[:][:, a:b]
        qmn, qmx = qv(0), qv(1)
        # shared pair = (O, E_sh) = rows (2p+1, 2p+2)
        tt(qmn, sv(O), sv(E_sh), op=MIN)
        tt(qmx, sv(O), sv(E_sh), op=MAX)
        # field completion order MN, MX, MD matches the ACT copy order in
        # _stage2_fused so no s1 copy ever stalls the DVE.
        # odd output rows r0+2p+1: pair + E; even rows: pair + O_sh2
        tt(f(MN2, 0), qmn, sv(E), op=MIN)
        tt(f(MN2, 1), qmn, sv(O_sh2), op=MIN)
        tt(f(MX2, 0), qmx, sv(E), op=MAX)
        tt(f(MX2, 1), qmx, sv(O_sh2), op=MAX)
        tt(tv(0), qmx, sv(E), op=MIN)
        tt(f(MD2, 0), qmn, tv(0), op=MAX)
        tt(tv(1), qmx, sv(O_sh2), op=MIN)
        tt(f(MD2, 1), qmn, tv(1), op=MAX)

    if first:
        for ia, ib in ((0, 1), (1, 3), (3, 6)):
            loads(ia, ib)
            stage1(ia, ib)
    else:
        loads(0, GIMG)
        stage1(0, GIMG)

    OUT = pio.tile([P, FP2], BF16, tag="OUT", name="OUT")
    _stage2_fused(nc, pm, MN2, MD2, MX2, OUT)

    out_img = lambda r_lo: oh[r_lo : min(r_lo + 2 * P, H) : 2, i0 : i0 + GIMG, :]
    osv = _seg(OUT, P, NSEG)
    if last:
        # HWDGE queues are idle by now (all loads issued); their stores
        # drain while the edge pass runs, shrinking the end-of-kernel tail
        nc.sync.dma_start(out_img(r0 + 1), osv[:, 0:GIMG, 1:513])
        nc.scalar.dma_start(out_img(r0 + 2), osv[:, GIMG:NSEG, 1:513])
    else:
        # stores on the SWDGE queue so they never block later blocks' loads
        nc.gpsimd.dma_start(out_img(r0 + 1), osv[:, 0:GIMG, 1:513])
        nc.gpsimd.dma_start(out_img(r0 + 2), osv[:, GIMG:NSEG, 1:513])


def _edge_loads(nc, pio, xi):
    """Loads for image rows 0 and 511 (tiny, partial-partition): issued up
    front so the end-of-kernel edge compute never waits on DMA."""
    R0 = pio.tile([NE, PW], BF16, tag="R0", name="R0")
    R1 = pio.tile([NE, PW], BF16, tag="R1", name="R1")
    for T in (R0, R1):
        nc.gpsimd.memset(T[:][0:NE, W:PW], 0.0)
    nc.sync.dma_start(R0[:][0:NIMG, 0:W], xi[:, 0, :])
    nc.scalar.dma_start(R1[:][0:NIMG, 0:W], xi[:, 1, :])
    nc.sync.dma_start(R0[:][NIMG:NE, 0:W], xi[:, H - 1, :])
    nc.scalar.dma_start(R1[:][NIMG:NE, 0:W], xi[:, H - 2, :])
    return R0, R1


def _edge_compute(nc, pio, pm, oi, R0, R1):
    """Rows 0 and 511 (windows contain the zero pad row), 24 partitions:
    p 0..11 = row 0 of image p; p 12..23 = row 511 of image p-12. Runs
    last, in the shadow of the final block's output stores."""
    rmn = pm.tile([NE, PW], BF16, tag="rmn", name="rmn")
    rmx = pm.tile([NE, PW], BF16, tag="rmx", name="rmx")
    nc.vector.tensor_tensor(rmn[:], R0[:], R1[:], op=MIN)
    nc.vector.tensor_tensor(rmx[:], R0[:], R1[:], op=MAX)

    # vertical sort3 with the zero pad row: min/max vs 0, med=max(mn,min(mx,0))
    MN0 = pm.tile([NE, PW], BF16, tag="eMN", name="eMN")
    MD0 = pm.tile([NE, PW], BF16, tag="eMD", name="eMD")
    MX0 = pm.tile([NE, PW], BF16, tag="eMX", name="eMX")
    nc.vector.tensor_scalar_min(MN0[:], rmn[:], 0.0)
    nc.vector.tensor_scalar_max(MX0[:], rmx[:], 0.0)
    nc.vector.scalar_tensor_tensor(MD0[:], rmx[:], 0.0, rmn[:], op0=MIN, op1=MAX)

    s1 = {}
    for name, F in (("MN", MN0), ("MD", MD0), ("MX", MX0)):
        T = pm.tile([NE, PW], BF16, tag=f"es1{name}", name=f"es1{name}")
        nc.scalar.copy(T[:][0:NE, 0 : PW - 1], F[:][0:NE, 1:PW])
        s1[name] = T

    OUT0 = pio.tile([NE, PW], BF16, tag="OUT0", name="OUT0")
    _stage2_compute_small(nc, pm, MN0, MD0, MX0, s1, OUT0)
    # the kernel's very last stores: HWDGE (faster first-byte than SWDGE)
    nc.sync.dma_start(oi[:, 0, :], OUT0[:][0:NIMG, 1:513])
    nc.scalar.dma_start(oi[:, H - 1, :], OUT0[:][NIMG:NE, 1:513])


def _stage2_compute_small(nc, pm, MN, MD, MX, s1, OUT):
    """Same dataflow as _stage2_fused on [NE, PW] tiles (one segment)."""
    NF = PW
    NI = NF - 2
    fl = lambda T, a, b: T[:][0:NE, a:b]

    def t2(tag):
        return pm.tile([NE, PW], BF16, tag=f"e{tag}", name=f"e{tag}")

    Pmn, Pmx, Qmn, Qmx = t2("Pmn"), t2("Pmx"), t2("Qmn"), t2("Qmx")
    tmd, Rmn, Rmd, Rmx = t2("tmd"), t2("Rmn"), t2("Rmd"), t2("Rmx")
    MN1, MX1, TF = t2("MN1"), t2("MX1"), t2("TF")

    tt = nc.vector.tensor_tensor
    tt(fl(Pmn, 0, NI), fl(MN, 0, NI), fl(s1["MN"], 0, NI), op=MAX)
    tt(fl(Qmn, 0, NI), fl(MD, 0, NI), fl(s1["MD"], 0, NI), op=MIN)
    tt(fl(Qmx, 0, NI), fl(MD, 0, NI), fl(s1["MD"], 0, NI), op=MAX)
    tt(fl(Pmx, 0, NI), fl(MX, 0, NI), fl(s1["MX"], 0, NI), op=MIN)
    tt(fl(Rmn, 0, NI), fl(Pmn, 0, NI), fl(MN, 2, NF), op=MAX)
    tt(fl(tmd, 0, NI), fl(Qmx, 0, NI), fl(MD, 2, NF), op=MIN)
    tt(fl(Rmd, 0, NI), fl(Qmn, 0, NI), fl(tmd, 0, NI), op=MAX)
    tt(fl(Rmx, 0, NI), fl(Pmx, 0, NI), fl(MX, 2, NF), op=MIN)
    tt(fl(MN1, 0, NI), fl(Rmn, 0, NI), fl(Rmd, 0, NI), op=MIN)
    tt(fl(MX1, 0, NI), fl(Rmn, 0, NI), fl(Rmd, 0, NI), op=MAX)
    tt(fl(TF, 0, NI), fl(MX1, 0, NI), fl(Rmx, 0, NI), op=MIN)
    tt(fl(OUT, 2, 512), fl(MN1, 0, 510), fl(TF, 0, 510), op=MAX)

    gv = lambda T: T[:][0:NE, 0:511:510]
    bt = lambda tag: pm.tile([NE, 2], BF16, tag=f"e{tag}b", name=f"e{tag}b")
    BA, BB, BC = bt("BA"), bt("BB"), bt("BC")
    B1, B2, B3 = bt("B1"), bt("B2"), bt("B3")
    nc.vector.tensor_scalar_max(BA[:], gv(Pmn), 0.0)
    nc.vector.tensor_scalar_min(BC[:], gv(Pmx), 0.0)
    nc.vector.scalar_tensor_tensor(BB[:], gv(Qmx), 0.0, gv(Qmn), op0=MIN, op1=MAX)
    tt(B1[:], BA[:], BB[:], op=MIN)
    tt(B2[:], BA[:], BB[:], op=MAX)
    tt(B3[:], B2[:], BC[:], op=MIN)
    tt(OUT[:][0:NE, 1:513:511], B1[:], B3[:], op=MAX)


def build_program():
    nc = bacc.Bacc(
        "TRN2", target_bir_lowering=False, debug=False, num_devices=N_CORES
    )
    x_d = nc.dram_tensor("x", [B_PER, C, H, W], BF16, kind="ExternalInput").ap()
    o_d = nc.dram_tensor("out", [B_PER, C, H, W], BF16, kind="ExternalOutput").ap()
    xh = x_d.rearrange("b c h w -> h (b c) w")  # [512, 12, 512]
    oh = o_d.rearrange("b c h w -> h (b c) w")
    xi = x_d.rearrange("b c h w -> (b c) h w")  # [12, 512, 512]
    oi = o_d.rearrange("b c h w -> (b c) h w")

    with tile.TileContext(nc) as tc:
        with (
            tc.tile_pool(name="io", bufs=1) as pio,
            tc.tile_pool(name="mid", bufs=1) as pm,
        ):
            nb = 2 * (NIMG // GIMG)
            _block(nc, pio, pm, xh, oh, 0, 0, first=True)
            # edge loads are tiny; edge COMPUTE runs last, in the shadow of
            # the final block's output stores
            R0, R1 = _edge_loads(nc, pio, xi)
            for i in range(1, nb):
                _block(nc, pio, pm, xh, oh, i // 2, i % 2, last=(i == nb - 1))
            _edge_compute(nc, pio, pm, oi, R0, R1)
    nc.compile()
    return nc


def _get_program():
    global _PROGRAM
    if _PROGRAM is None:
        _PROGRAM = build_program()
    return _PROGRAM


def make_in_maps(x: np.ndarray):
    xb = np.ascontiguousarray(x).astype(ml_dtypes.bfloat16)
    return [{"x": xb[k * B_PER : (k + 1) * B_PER]} for k in range(N_CORES)]


def kernel(**inputs) -> np.ndarray:
    x = np.asarray(inputs["x"], dtype=np.float32)
    assert x.shape == (B, C, H, W), x.shape
    nc = _get_program()
    res = bass_utils.run_bass_kernel_spmd(
        nc, make_in_maps(x), core_ids=list(range(N_CORES))
    )
    out = np.concatenate(
        [np.asarray(res.results[k]["out"]) for k in range(N_CORES)], axis=0
    )
    return out.astype(np.float32)


# revision 18
# speedup vs baseline: 1.0155x; 1.0092x over previous
"""3x3 median filter (zero-padded) on TRN2, 8 NeuronCores, bf16 datapath.

Input  x: (32, 3, 512, 512) float32
Output  : (32, 3, 512, 512) float32.

Accuracy: the median network only ever SELECTS one of its 9 inputs (min/max
ops create no new values), so the device-side bf16 result equals the bf16
rounding of the element that is the median of the rounded window. Order
statistics are 1-Lipschitz under sup-norm perturbation, so end-to-end error
is <= 2^-8 relative -- far inside the 2e-2 gate. Measured 3.4e-3.

Strategy
--------
Pure data parallel: batch dim sharded 4-per-core across 8 cores; per core
12 images (4 batch x 3 chan) in 2 groups of 6 images x 2 vertical halves.

bf16 doubles DVE tensor_tensor throughput (2x_1P perf mode) but ONLY for
unit-stride 4-byte-aligned access patterns, so the horizontal stage is
restructured from the fp32 baseline's stride-2 parity tricks into dense
shifted-field form. Per field F in {MN,MD,MX}: s1F[j]=F[j+1] is the ONLY
odd-element shift (a ScalarE copy -- ACT is otherwise idle), then
   P[j] = op(F[j], s1F[j])          # aligned TT, 2x
   R[j] = op(P[j], F[j+2])          # +2 elems = 4B-aligned shift, 2x
R[j] = sliding-3 result centered at col j+1; the final med3(Rmn,Rmd,Rmx)
writes into an OUT grid whose per-image segment holds col c at position
c+1, so the wide write starts at even offset 2 and the DMA store (which
doesn't care about alignment) un-shifts.

Both row parities' fields live in ONE fused [128, 12*514] tile (odd-parity
images = segments 0..5, even = 6..11), so stage 2 is 12 double-width ops
per block instead of 24 -- per-op overhead (58-cycle issue + ~90ns DRAIN)
is the only thing that changes, the streamed cycles are identical. Output
cols 0 and 511 (windows containing the zero pad column) are 7 tiny ops on
gathered grid positions {0,510} across all 12 segments.

Grid: per-image segment width 514 (even -> every segment start keeps 4B
parity). Segment positions 512..513 are scratch: stage-1 ops run flat over
the whole grid and compute garbage there; no stored output reads a garbage
lane (out cols 0/511 come from the boundary path).

SBUF fits via aggressive aliasing with DVE-program-order-safe lifetimes:
stage-1's qmn/qmx/t_o/t_e live in halves of stage-2's Pmx/Qmx slots;
stage-2's Rmx/tmd/Rmd overwrite the dead MN/MX/MD field buffers; MN1/MX1/TF
overwrite Pmn/Qmn/Qmx. Stage-1 emits fields in MN,MX,MD order and ACT
copies s1MN,s1MX,s1MD in that order so every copy lands before the DVE
needs it, with no stall.

Image rows 0 and 511 (windows contain the zero pad row): one small
24-partition pass issued LAST so it fills the DVE-idle tail while the
final block's output stores (sent to the idle HWDGE queues) drain. Its
tiny loads are issued up front. Block 0's loads+stage-1 ramp up in 1+2+3
image chunks sized to the ~250GB/s strided-row-gather DMA rate, so the
DVE starts ~2us after the first 0.25MB lands instead of waiting for 3MB.

Engine budget per core: DVE 17 TT/elem at 2x ~= 236us busy (the floor for
this decomposition), ACT ~75us, DMA ~19MB. Loads on the SP+ACT HWDGE
queues, mid-kernel stores on the GpSimd SWDGE queue.
"""
import sys

if "/opt/trn_rl_repo" not in sys.path:
    sys.path.insert(0, "/opt/trn_rl_repo")

import numpy as np
import ml_dtypes
import concourse.bacc as bacc
import concourse.mybir as mybir
import concourse.tile as tile
from concourse import bass_utils

B, C, H, W = 32, 3, 512, 512
N_CORES = 8
B_PER = B // N_CORES          # 4 batches per core
NIMG = B_PER * C              # 12 images per core
GIMG = 6                      # images per tile group
PW = W + 2                    # per-image grid width (514, even)
FP = GIMG * PW                # half (one parity) grid width (3084)
FP2 = 2 * FP                  # fused two-parity grid width (6168)
NSEG = 2 * GIMG               # segments in a fused tile (12)
HH = H // 2                   # 256 rows per vertical half
P = 128                       # partitions = row pairs per half
NE = 2 * NIMG                 # partitions in the edge-rows pass (24)

BF16 = mybir.dt.bfloat16
MIN = mybir.AluOpType.min
MAX = mybir.AluOpType.max

_PROGRAM = None


def _seg(T, npart, nseg):
    """[npart, nseg, 514] per-image-segment view."""
    return T[:].rearrange("p (i w) -> p i w", w=PW)[0:npart, 0:nseg]


def _stage2_fused(nc, pm, MN, MD, MX, OUT):
    """Both parities of stage 2 in double-width ops on the fused grids.
    ScalarE does the three s1 shifts (issued in MN,MX,MD order to match
    the DVE's consumption order). Aliases overwrite only dead buffers."""
    NI = FP2 - 2
    fl = lambda T, a, b: T[:][:, a:b]

    s1 = {}
    for nm, F in (("MN", MN), ("MX", MX), ("MD", MD)):
        T = pm.tile([P, FP2], BF16, tag=f"s1{nm}", name=f"s1{nm}")
        nc.scalar.copy(T[:][:, 0 : FP2 - 1], F[:][:, 1:FP2])
        s1[nm] = T

    def t2(tag):
        return pm.tile([P, FP2], BF16, tag=tag, name=tag)

    def alias(tag, name):
        return pm.tile([P, FP2], BF16, tag=tag, name=name)

    Pmn, Pmx, Qmn, Qmx, Rmn = t2("Pmn"), t2("Pmx"), t2("Qmn"), t2("Qmx"), t2("Rmn")
    Rmx = alias("fMN", "Rmx")   # MN field dead after Rmn
    tmd = alias("fMX", "tmd")   # MX field dead after Rmx
    Rmd = alias("fMD", "Rmd")   # MD field dead after tmd
    MN1 = alias("Pmn", "MN1")
    MX1 = alias("Qmn", "MX1")
    TF = alias("Qmx", "TF")

    tt = nc.vector.tensor_tensor
    gv = lambda T: _seg(T, P, NSEG)[:, :, 0:511:510]
    bt = lambda tag: pm.tile([P, 2 * NSEG], BF16, tag=f"{tag}b", name=f"{tag}b")
    bv = lambda T: T[:].rearrange("p (i c) -> p i c", c=2)[0:P, 0:NSEG]
    BA, BB, BC = bt("BA"), bt("BB"), bt("BC")
    B1, B2, B3 = bt("B1"), bt("B2"), bt("B3")

    tt(fl(Pmn, 0, NI), fl(MN, 0, NI), fl(s1["MN"], 0, NI), op=MAX)
    tt(fl(Pmx, 0, NI), fl(MX, 0, NI), fl(s1["MX"], 0, NI), op=MIN)
    nc.vector.tensor_scalar_max(bv(BA), gv(Pmn), 0.0)
    nc.vector.tensor_scalar_min(bv(BC), gv(Pmx), 0.0)
    tt(fl(Rmn, 0, NI), fl(Pmn, 0, NI), fl(MN, 2, FP2), op=MAX)
    tt(fl(Rmx, 0, NI), fl(Pmx, 0, NI), fl(MX, 2, FP2), op=MIN)
    tt(fl(Qmn, 0, NI), fl(MD, 0, NI), fl(s1["MD"], 0, NI), op=MIN)
    tt(fl(Qmx, 0, NI), fl(MD, 0, NI), fl(s1["MD"], 0, NI), op=MAX)
    nc.vector.scalar_tensor_tensor(bv(BB), gv(Qmx), 0.0, gv(Qmn), op0=MIN, op1=MAX)
    tt(fl(tmd, 0, NI), fl(Qmx, 0, NI), fl(MD, 2, FP2), op=MIN)
    tt(fl(Rmd, 0, NI), fl(Qmn, 0, NI), fl(tmd, 0, NI), op=MAX)
    # med3(Rmn, Rmd, Rmx); last wide op writes output cols 1..510 at grid
    # positions 2..511 (even start -> stays in 2x mode)
    tt(fl(MN1, 0, NI), fl(Rmn, 0, NI), fl(Rmd, 0, NI), op=MIN)
    tt(fl(MX1, 0, NI), fl(Rmn, 0, NI), fl(Rmd, 0, NI), op=MAX)
    tt(fl(TF, 0, NI), fl(MX1, 0, NI), fl(Rmx, 0, NI), op=MIN)
    ov = _seg(OUT, P, NSEG)[:, :, 2:512]
    tt(ov, _seg(MN1, P, NSEG)[:, :, 0:510], _seg(TF, P, NSEG)[:, :, 0:510],
       op=MAX)
    # boundary med3 -> OUT positions {1, 512}
    tt(bv(B1), bv(BA), bv(BB), op=MIN)
    tt(bv(B2), bv(BA), bv(BB), op=MAX)
    tt(bv(B3), bv(B2), bv(BC), op=MIN)
    tt(_seg(OUT, P, NSEG)[:, :, 1:513:511], bv(B1), bv(B3), op=MAX)


def _block(nc, pio, pm, xh, oh, g, half, first=False, last=False):
    """One vertical half of one image group: odd output rows r0+1..r0+255,
    even rows r0+2..r0+256 (halves overlap by two rows so every DMA is a
    full 128-partition transfer). Rows 0 and 511 via the edge pass.
    first=True: loads and stage 1 ramp in 1+2+3 image chunks (cold start).
    last=True: stores go to the by-then-idle HWDGE queues so the final
    drain overlaps the edge pass."""
    r0 = 0 if half == 0 else H - HH - 2
    i0 = GIMG * g

    E = pio.tile([P, FP], BF16, tag="E", name="E")
    O = pio.tile([P, FP], BF16, tag="O", name="O")
    E_sh = pio.tile([P, FP], BF16, tag="E_sh", name="E_sh")
    O_sh2 = pio.tile([P, FP], BF16, tag="O_sh2", name="O_sh2")

    # scratch cols 512..513 of each segment are read by the flat stage-1
    # ops but never loaded: define them so no lane is uninitialized
    for T in (E, O, E_sh, O_sh2):
        nc.gpsimd.memset(_seg(T, P, GIMG)[:, :, W:PW], 0.0)

    def loads(ia, ib):
        lv = lambda T: _seg(T, P, GIMG)[:, ia:ib, 0:W]
        im = lambda r_lo: xh[
            r_lo : min(r_lo + 2 * P, H) : 2, i0 + ia : i0 + ib, :
        ]
        # queue order matters (HWDGE queues are FIFOs): the (O, E_sh) pair
        # feeds the first op of the block, so those loads go first
        nc.sync.dma_start(lv(E_sh), im(r0 + 2))     # rows r0+2p+2
        nc.scalar.dma_start(lv(O), im(r0 + 1))      # rows r0+2p+1
        nc.sync.dma_start(lv(E), im(r0))            # rows r0+2p
        nc.scalar.dma_start(lv(O_sh2), im(r0 + 3))  # rows r0+2p+3

    # stage-1 temps live in halves of stage-2 slots that are written later
    qpair = pm.tile([P, FP2], BF16, tag="Pmx", name="qpair")
    tpair = pm.tile([P, FP2], BF16, tag="Qmx", name="tpair")
    MN2 = pm.tile([P, FP2], BF16, tag="fMN", name="fMN")
    MD2 = pm.tile([P, FP2], BF16, tag="fMD", name="fMD")
    MX2 = pm.tile([P, FP2], BF16, tag="fMX", name="fMX")

    tt = nc.vector.tensor_tensor

    def stage1(ia, ib):
        a, b = ia * PW, ib * PW
        qv = lambda h: qpair[:][:, h * FP + a : h * FP + b]
        tv = lambda h: tpair[:][:, h * FP + a : h * FP + b]
        f = lambda T, h: T[:][:, h * FP + a : h * FP + b]
        sv = lambda T: T[:][:, a:b]
        qmn, qmx = qv(0), qv(1)
        # shared pair = (O, E_sh) = rows (2p+1, 2p+2)
        tt(qmn, sv(O), sv(E_sh), op=MIN)
        tt(qmx, sv(O), sv(E_sh), op=MAX)
        # field completion order MN, MX, MD matches the ACT copy order in
        # _stage2_fused so no s1 copy ever stalls the DVE.
        # odd output rows r0+2p+1: pair + E; even rows: pair + O_sh2
        tt(f(MN2, 0), qmn, sv(E), op=MIN)
        tt(f(MN2, 1), qmn, sv(O_sh2), op=MIN)
        tt(f(MX2, 0), qmx, sv(E), op=MAX)
        tt(f(MX2, 1), qmx, sv(O_sh2), op=MAX)
        tt(tv(0), qmx, sv(E), op=MIN)
        tt(f(MD2, 0), qmn, tv(0), op=MAX)
        tt(tv(1), qmx, sv(O_sh2), op=MIN)
        tt(f(MD2, 1), qmn, tv(1), op=MAX)

    if first:
        for ia, ib in ((0, 1), (1, 3), (3, 6)):
            loads(ia, ib)
            stage1(ia, ib)
    else:
        loads(0, GIMG)
        stage1(0, GIMG)

    OUT = pio.tile([P, FP2], BF16, tag="OUT", name="OUT")
    _stage2_fused(nc, pm, MN2, MD2, MX2, OUT)

    out_img = lambda r_lo: oh[r_lo : min(r_lo + 2 * P, H) : 2, i0 : i0 + GIMG, :]
    osv = _seg(OUT, P, NSEG)
    if last:
        # HWDGE queues are idle by now (all loads issued); their stores
        # drain while the edge pass runs, shrinking the end-of-kernel tail
        nc.sync.dma_start(out_img(r0 + 1), osv[:, 0:GIMG, 1:513])
        nc.scalar.dma_start(out_img(r0 + 2), osv[:, GIMG:NSEG, 1:513])
    else:
        # stores on the SWDGE queue so they never block later blocks' loads
        nc.gpsimd.dma_start(out_img(r0 + 1), osv[:, 0:GIMG, 1:513])
        nc.gpsimd.dma_start(out_img(r0 + 2), osv[:, GIMG:NSEG, 1:513])


def _edge_loads(nc, pio, xi):
    """Loads for image rows 0 and 511 (tiny, partial-partition): issued up
    front so the end-of-kernel edge compute never waits on DMA."""
    R0 = pio.tile([NE, PW], BF16, tag="R0", name="R0")
    R1 = pio.tile([NE, PW], BF16, tag="R1", name="R1")
    for T in (R0, R1):
        nc.gpsimd.memset(T[:][0:NE, W:PW], 0.0)
    nc.sync.dma_start(R0[:][0:NIMG, 0:W], xi[:, 0, :])
    nc.scalar.dma_start(R1[:][0:NIMG, 0:W], xi[:, 1, :])
    nc.sync.dma_start(R0[:][NIMG:NE, 0:W], xi[:, H - 1, :])
    nc.scalar.dma_start(R1[:][NIMG:NE, 0:W], xi[:, H - 2, :])
    return R0, R1


def _edge_compute(nc, pio, pm, oi, R0, R1):
    """Rows 0 and 511 (windows contain the zero pad row), 24 partitions:
    p 0..11 = row 0 of image p; p 12..23 = row 511 of image p-12. Runs
    last, in the shadow of the final block's output stores."""
    rmn = pm.tile([NE, PW], BF16, tag="rmn", name="rmn")
    rmx = pm.tile([NE, PW], BF16, tag="rmx", name="rmx")
    nc.vector.tensor_tensor(rmn[:], R0[:], R1[:], op=MIN)
    nc.vector.tensor_tensor(rmx[:], R0[:], R1[:], op=MAX)

    # vertical sort3 with the zero pad row: min/max vs 0, med=max(mn,min(mx,0))
    MN0 = pm.tile([NE, PW], BF16, tag="eMN", name="eMN")
    MD0 = pm.tile([NE, PW], BF16, tag="eMD", name="eMD")
    MX0 = pm.tile([NE, PW], BF16, tag="eMX", name="eMX")
    nc.vector.tensor_scalar_min(MN0[:], rmn[:], 0.0)
    nc.vector.tensor_scalar_max(MX0[:], rmx[:], 0.0)
    nc.vector.scalar_tensor_tensor(MD0[:], rmx[:], 0.0, rmn[:], op0=MIN, op1=MAX)

    s1 = {}
    for name, F in (("MN", MN0), ("MD", MD0), ("MX", MX0)):
        T = pm.tile([NE, PW], BF16, tag=f"es1{name}", name=f"es1{name}")
        nc.scalar.copy(T[:][0:NE, 0 : PW - 1], F[:][0:NE, 1:PW])
        s1[name] = T

    OUT0 = pio.tile([NE, PW], BF16, tag="OUT0", name="OUT0")
    _stage2_compute_small(nc, pm, MN0, MD0, MX0, s1, OUT0)
    # SWDGE is empty by now (its last work was the mid-kernel stores), so
    # these tiny final stores' completion overlaps the HWDGE stores' ack
    nc.gpsimd.dma_start(oi[:, 0, :], OUT0[:][0:NIMG, 1:513])
    nc.gpsimd.dma_start(oi[:, H - 1, :], OUT0[:][NIMG:NE, 1:513])


def _stage2_compute_small(nc, pm, MN, MD, MX, s1, OUT):
    """Same dataflow as _stage2_fused on [NE, PW] tiles (one segment)."""
    NF = PW
    NI = NF - 2
    fl = lambda T, a, b: T[:][0:NE, a:b]

    def t2(tag):
        return pm.tile([NE, PW], BF16, tag=f"e{tag}", name=f"e{tag}")

    Pmn, Pmx, Qmn, Qmx = t2("Pmn"), t2("Pmx"), t2("Qmn"), t2("Qmx")
    tmd, Rmn, Rmd, Rmx = t2("tmd"), t2("Rmn"), t2("Rmd"), t2("Rmx")
    MN1, MX1, TF = t2("MN1"), t2("MX1"), t2("TF")

    tt = nc.vector.tensor_tensor
    tt(fl(Pmn, 0, NI), fl(MN, 0, NI), fl(s1["MN"], 0, NI), op=MAX)
    tt(fl(Qmn, 0, NI), fl(MD, 0, NI), fl(s1["MD"], 0, NI), op=MIN)
    tt(fl(Qmx, 0, NI), fl(MD, 0, NI), fl(s1["MD"], 0, NI), op=MAX)
    tt(fl(Pmx, 0, NI), fl(MX, 0, NI), fl(s1["MX"], 0, NI), op=MIN)
    tt(fl(Rmn, 0, NI), fl(Pmn, 0, NI), fl(MN, 2, NF), op=MAX)
    tt(fl(tmd, 0, NI), fl(Qmx, 0, NI), fl(MD, 2, NF), op=MIN)
    tt(fl(Rmd, 0, NI), fl(Qmn, 0, NI), fl(tmd, 0, NI), op=MAX)
    tt(fl(Rmx, 0, NI), fl(Pmx, 0, NI), fl(MX, 2, NF), op=MIN)
    tt(fl(MN1, 0, NI), fl(Rmn, 0, NI), fl(Rmd, 0, NI), op=MIN)
    tt(fl(MX1, 0, NI), fl(Rmn, 0, NI), fl(Rmd, 0, NI), op=MAX)
    tt(fl(TF, 0, NI), fl(MX1, 0, NI), fl(Rmx, 0, NI), op=MIN)
    tt(fl(OUT, 2, 512), fl(MN1, 0, 510), fl(TF, 0, 510), op=MAX)

    gv = lambda T: T[:][0:NE, 0:511:510]
    bt = lambda tag: pm.tile([NE, 2], BF16, tag=f"e{tag}b", name=f"e{tag}b")
    BA, BB, BC = bt("BA"), bt("BB"), bt("BC")
    B1, B2, B3 = bt("B1"), bt("B2"), bt("B3")
    nc.vector.tensor_scalar_max(BA[:], gv(Pmn), 0.0)
    nc.vector.tensor_scalar_min(BC[:], gv(Pmx), 0.0)
    nc.vector.scalar_tensor_tensor(BB[:], gv(Qmx), 0.0, gv(Qmn), op0=MIN, op1=MAX)
    tt(B1[:], BA[:], BB[:], op=MIN)
    tt(B2[:], BA[:], BB[:], op=MAX)
    tt(B3[:], B2[:], BC[:], op=MIN)
    tt(OUT[:][0:NE, 1:513:511], B1[:], B3[:], op=MAX)


def build_program():
    nc = bacc.Bacc(
        "TRN2", target_bir_lowering=False, debug=False, num_devices=N_CORES
    )
    x_d = nc.dram_tensor("x", [B_PER, C, H, W], BF16, kind="ExternalInput").ap()
    o_d = nc.dram_tensor("out", [B_PER, C, H, W], BF16, kind="ExternalOutput").ap()
    xh = x_d.rearrange("b c h w -> h (b c) w")  # [512, 12, 512]
    oh = o_d.rearrange("b c h w -> h (b c) w")
    xi = x_d.rearrange("b c h w -> (b c) h w")  # [12, 512, 512]
    oi = o_d.rearrange("b c h w -> (b c) h w")

    with tile.TileContext(nc) as tc:
        with (
            tc.tile_pool(name="io", bufs=1) as pio,
            tc.tile_pool(name="mid", bufs=1) as pm,
        ):
            nb = 2 * (NIMG // GIMG)
            _block(nc, pio, pm, xh, oh, 0, 0, first=True)
            # edge loads are tiny; edge COMPUTE runs last, in the shadow of
            # the final block's output stores
            R0, R1 = _edge_loads(nc, pio, xi)
            for i in range(1, nb):
                _block(nc, pio, pm, xh, oh, i // 2, i % 2, last=(i == nb - 1))
            _edge_compute(nc, pio, pm, oi, R0, R1)
    nc.compile()
    return nc


def _get_program():
    global _PROGRAM
    if _PROGRAM is None:
        _PROGRAM = build_program()
    return _PROGRAM


def make_in_maps(x: np.ndarray):
    xb = np.ascontiguousarray(x).astype(ml_dtypes.bfloat16)
    return [{"x": xb[k * B_PER : (k + 1) * B_PER]} for k in range(N_CORES)]


def kernel(**inputs) -> np.ndarray:
    x = np.asarray(inputs["x"], dtype=np.float32)
    assert x.shape == (B, C, H, W), x.shape
    nc = _get_program()
    res = bass_utils.run_bass_kernel_spmd(
        nc, make_in_maps(x), core_ids=list(range(N_CORES))
    )
    out = np.concatenate(
        [np.asarray(res.results[k]["out"]) for k in range(N_CORES)], axis=0
    )
    return out.astype(np.float32)
